# revision 1
# baseline (speedup 1.0000x reference)
"""BiMamba Trainium2 kernel.

Sharding: 8 cores = (batch 4) x (d-half 2). Every core runs the same SPMD
program: full input projection + depthwise conv + delta/B/C projections
(over all 512 internal channels), then the bidirectional selective scan for
its 256-channel d-half (both directions), gating, and a partial output
projection. The host sums the two partial outputs per batch element.

The d-axis of all weights is permuted per core so that the core's d-half
always occupies channels 0..255 — this keeps the program identical across
cores (pure SPMD, only the input data differs). Inputs are passed
pre-transposed ([d_in, l]) and the depthwise conv kernel as per-tile
diagonal matrices so the conv runs as PSUM-accumulated matmuls.

Scan: h[t] = exp(dA[t]) * h[t-1] + delta*u*B[t] via the DVE
tensor_tensor_scan primitive, two states per instruction (the decay is
zeroed at the n-block boundary, which is exact because the recurrence
starts from 0). The backward direction runs the same scan through
negative-stride access patterns with B/C rows time-flipped, letting both
directions share the per-step decay exp(A*delta) up to boundary columns.
The 16 per-state readout products are accumulated into PSUM with identity
matmuls on the tensor engine.
"""

import sys

for _p in ("/opt/trn_rl_repo",):
    if _p not in sys.path:
        sys.path.insert(0, _p)

from contextlib import ExitStack

import numpy as np

B_SZ, L, D_IN, D_INT = 4, 1024, 256, 512
N_ST, DTR, D_CONV = 16, 16, 4
P = 128
DH = D_INT // 2        # d channels per core (256)
NDT = DH // P          # d-tiles per core in the scan (2)
N_CORES = 8

_cache = {}


class TileCtx:
    """TileContext plus an ExitStack closed before the context exits."""

    def __init__(self, tile_mod, nc):
        self._tc = tile_mod.TileContext(nc)
        self._st = ExitStack()

    def __enter__(self):
        tc = self._tc.__enter__()
        return tc, self._st

    def __exit__(self, *exc):
        self._st.close()
        return self._tc.__exit__(*exc)


def _build_program():
    import concourse.bacc as bacc
    import concourse.tile as tile
    import concourse.mybir as mybir
    from concourse import masks

    dt = mybir.dt
    ST = dt.bfloat16
    f32r = dt.float32r
    Alu = mybir.AluOpType
    AF = mybir.ActivationFunctionType

    nc = bacc.Bacc()

    inpT_d = nc.dram_tensor("inpT", (D_IN, L), dt.float32, kind="ExternalInput")
    w_in_d = nc.dram_tensor("w_in", (D_IN, 4 * D_IN), dt.float32, kind="ExternalInput")
    ckd_d = nc.dram_tensor("ckd", (4, D_CONV, P, P), dt.float32, kind="ExternalInput")
    cb_d = nc.dram_tensor("cb", (D_INT, 1), dt.float32, kind="ExternalInput")
    w_x_d = nc.dram_tensor("w_x", (D_INT, DTR + 4 * N_ST), dt.float32, kind="ExternalInput")
    w_dt_d = nc.dram_tensor("w_dt", (DTR, DH), dt.float32, kind="ExternalInput")
    bdt_d = nc.dram_tensor("bdt", (DH, 1), dt.float32, kind="ExternalInput")
    a_d = nc.dram_tensor("a", (DH, N_ST), dt.float32, kind="ExternalInput")
    dpar_d = nc.dram_tensor("dpar", (DH, 1), dt.float32, kind="ExternalInput")
    w_out_d = nc.dram_tensor("w_out", (4, P, D_IN), dt.float32, kind="ExternalInput")
    out_d = nc.dram_tensor("out_part", (L, D_IN), dt.float32, kind="ExternalOutput")

    NLC = L // P           # l-chunks (8)
    NKT = D_IN // P        # k-tiles of the input dim (2)
    NX = DTR + 4 * N_ST    # x_dbl rows (80)
    J_X = list(range(4))   # x_and_res column tiles: x part
    J_R = [4, 5]           # res tiles of our (permuted-to-front) d-half

    with TileCtx(tile, nc) as (tc, st):
        cpool = st.enter_context(tc.tile_pool(name="consts", bufs=1))
        main = st.enter_context(tc.tile_pool(name="main", bufs=1))
        drp = st.enter_context(tc.tile_pool(name="dr", bufs=1, space="DRAM"))
        scratch = drp.tile([4 * N_ST, L], ST, name="scratch")

        # ---------------- constants / weights ----------------
        ident16 = cpool.tile([P, P], ST, name="ident16")
        masks.make_identity(nc, ident16[:])

        ckd_sb = [cpool.tile([P, D_CONV * P], f32r, name=f"ckd{t}", tag=f"ckd{t}")
                  for t in range(4)]
        cb_sb = [cpool.tile([P, 1], dt.float32, name=f"cb{t}", tag=f"cb{t}") for t in range(4)]
        w_x_sb = [cpool.tile([P, NX], dt.float32, name=f"wx{t}", tag=f"wx{t}") for t in range(4)]
        for t in range(4):
            nc.scalar.dma_start(cb_sb[t][:], cb_d[t * P:(t + 1) * P, :])
            nc.scalar.dma_start(w_x_sb[t][:], w_x_d[t * P:(t + 1) * P, :])
        w_dt_sb = cpool.tile([DTR, DH], dt.float32, name="w_dt_sb")
        nc.scalar.dma_start(w_dt_sb[:], w_dt_d[:])
        bdt_sb = [cpool.tile([P, 1], dt.float32, name=f"bdt{t}", tag=f"bdt{t}") for t in range(NDT)]
        a_sb = [cpool.tile([P, N_ST], dt.float32, name=f"a{t}", tag=f"a{t}") for t in range(NDT)]
        dpar_sb = [cpool.tile([P, 1], dt.float32, name=f"dp{t}", tag=f"dp{t}") for t in range(NDT)]
        for t in range(NDT):
            nc.scalar.dma_start(bdt_sb[t][:], bdt_d[t * P:(t + 1) * P, :])
            nc.scalar.dma_start(a_sb[t][:], a_d[t * P:(t + 1) * P, :])
            nc.scalar.dma_start(dpar_sb[t][:], dpar_d[t * P:(t + 1) * P, :])
        w_out_sb = [cpool.tile([P, D_IN], f32r, name=f"wo{t}", tag=f"wo{t}") for t in range(4)]

        # persistent activations (core's d-half only)
        xs = [main.tile([P, L], dt.float32, name=f"xs{t}", tag=f"xs{t}") for t in range(NDT)]
        sres = [main.tile([P, L], dt.float32, name=f"sres{i}", tag=f"sres{i}") for i in range(2)]
        delta = [main.tile([P, L], dt.float32, name=f"delta{t}", tag=f"delta{t}") for t in range(NDT)]
        zu = [main.tile([P, L], ST, name=f"zu{t}", tag=f"zu{t}") for t in range(NDT)]
        gated = {}
        for di in range(2):
            for t in range(NDT):
                gated[(di, t)] = main.tile([P, L], f32r, name=f"gated{di}{t}", tag=f"g8{di}{t}")

        # ============ phase 1: projections, conv, delta ============
        with (
            tc.tile_pool(name="pre", bufs=1) as pre,
            tc.tile_pool(name="tmp", bufs=2) as tmp,
            tc.tile_pool(name="psB", bufs=3, space="PSUM") as psB,
            tc.tile_pool(name="psC", bufs=2, space="PSUM") as psC,
        ):
            for t in range(4):
                wst = tmp.tile([P, D_IN], dt.float32, name="wst", tag="wst")
                nc.sync.dma_start(wst[:], w_out_d[t, :, :])
                nc.vector.tensor_copy(w_out_sb[t][:], wst[:])
                cst = tmp.tile([P, D_CONV * P], dt.float32, name="cst", tag="cst")
                nc.scalar.dma_start(cst[:].rearrange("p (w q) -> p w q", w=D_CONV),
                                    ckd_d[t, :, :, :].transpose([1, 0, 2]))
                nc.vector.tensor_copy(ckd_sb[t][:], cst[:])

            inpT0 = [pre.tile([P, L], dt.float32, name=f"inpT0{k}", tag=f"inpT0{k}") for k in range(NKT)]
            inpT = [pre.tile([P, L], f32r, name=f"inpT{k}", tag=f"inpT{k}") for k in range(NKT)]
            for k in range(NKT):
                nc.sync.dma_start(inpT0[k][:], inpT_d[k * P:(k + 1) * P, :])
                nc.vector.tensor_copy(inpT[k][:], inpT0[k][:])
            w_in0 = [pre.tile([P, 4 * D_IN], dt.float32, name=f"wi0{k}", tag=f"wi0{k}") for k in range(NKT)]
            w_in_sb = [pre.tile([P, 4 * D_IN], f32r, name=f"wi{k}", tag=f"wi{k}") for k in range(NKT)]
            for k in range(NKT):
                nc.sync.dma_start(w_in0[k][:], w_in_d[k * P:(k + 1) * P, :])
                nc.vector.tensor_copy(w_in_sb[k][:], w_in0[k][:])

            xpad = [pre.tile([P, L + 3], f32r, name=f"xpad{t}", tag=f"xpad{t}") for t in range(4)]
            for t in range(4):
                nc.vector.memset(xpad[t][:].bitcast(dt.float32), 0.0)
            xs_hi = [pre.tile([P, L], dt.float32, name=f"xsh{t}", tag=f"xsh{t}") for t in range(2)]
            xs_all = xs + xs_hi

            # x_and_res^T = W_in^T @ inputs^T   (fp32r matmuls)
            rT = [pre.tile([P, L], dt.float32, name=f"rT{i}", tag=f"rT{i}") for i in range(2)]
            for j in J_X + J_R:
                for lh in range(2):
                    mm = psB.tile([P, 512], dt.float32, name="mm", tag="mm")
                    for k in range(NKT):
                        nc.tensor.matmul(
                            mm[:], w_in_sb[k][:, j * P:(j + 1) * P],
                            inpT[k][:, lh * 512:(lh + 1) * 512],
                            start=(k == 0), stop=(k == NKT - 1))
                    if j in J_X:
                        nc.vector.tensor_copy(xpad[j][:, 1 + lh * 512:1 + (lh + 1) * 512], mm[:])
                    else:
                        nc.vector.tensor_copy(rT[j - 4][:, lh * 512:(lh + 1) * 512], mm[:])

            # depthwise conv as diag-matmuls; silu computed off PSUM
            for t in range(4):
                for lh in range(2):
                    cm = psC.tile([P, 512], dt.float32, name="cm", tag="cm")
                    for w in range(D_CONV):
                        nc.tensor.matmul(
                            cm[:], ckd_sb[t][:, w * P:(w + 1) * P],
                            xpad[t][:, w + lh * 512:w + lh * 512 + 512],
                            start=(w == 0), stop=(w == D_CONV - 1))
                    sg = tmp.tile([P, 512], dt.float32, name="sg", tag="sg")
                    nc.scalar.activation(sg[:], cm[:], AF.Sigmoid, bias=cb_sb[t][:], scale=1.0)
                    nc.vector.scalar_tensor_tensor(
                        xs_all[t][:, lh * 512:(lh + 1) * 512], cm[:], cb_sb[t][:], sg[:],
                        Alu.add, Alu.mult)

            # silu(res) for our d-half
            for i in range(2):
                sg = tmp.tile([P, L], dt.float32, name="sgr", tag="sgr")
                nc.scalar.activation(sg[:], rT[i][:], AF.Sigmoid)
                nc.vector.tensor_mul(sres[i][:], rT[i][:], sg[:])

            # x_dbl^T = W_x^T @ xs
            xdb = pre.tile([NX, L], dt.float32, name="xdb")
            for lh in range(2):
                mm = psB.tile([NX, 512], dt.float32, name="mmx", tag="mmx")
                for t in range(4):
                    nc.tensor.matmul(mm[:], w_x_sb[t][:],
                                     xs_all[t][:, lh * 512:(lh + 1) * 512],
                                     start=(t == 0), stop=(t == 3))
                nc.vector.tensor_copy(xdb[:, lh * 512:(lh + 1) * 512], mm[:])

            # delta = softplus(x_dbl[:, :16] @ W_dt + b_dt);  zu = delta * xs
            for t in range(NDT):
                for lh in range(2):
                    mm = psB.tile([P, 512], dt.float32, name="mm", tag="mm")
                    nc.tensor.matmul(mm[:], w_dt_sb[:, t * P:(t + 1) * P],
                                     xdb[0:DTR, lh * 512:(lh + 1) * 512],
                                     start=True, stop=True)
                    # softplus(pre + b_dt) = ln(1 + exp(pre + b_dt))
                    et = tmp.tile([P, 512], dt.float32, name="et", tag="et")
                    nc.scalar.activation(et[:], mm[:], AF.Exp, bias=bdt_sb[t][:], scale=1.0)
                    nc.scalar.activation(delta[t][:, lh * 512:(lh + 1) * 512], et[:],
                                         AF.Ln, bias=1.0, scale=1.0)
                nc.vector.tensor_mul(zu[t][:], delta[t][:], xs[t][:])

            # stage B/C rows to DRAM (bwd rows time-flipped) for broadcast
            ftmp = pre.tile([NX, L], dt.float32, name="ftmp")
            nc.vector.tensor_copy(ftmp[DTR + N_ST:DTR + 2 * N_ST, :],
                                  xdb[DTR + N_ST:DTR + 2 * N_ST, ::-1])
            nc.vector.tensor_copy(ftmp[DTR + 3 * N_ST:DTR + 4 * N_ST, :],
                                  xdb[DTR + 3 * N_ST:DTR + 4 * N_ST, ::-1])
            nc.gpsimd.dma_start(scratch[0:N_ST, :], xdb[DTR:DTR + N_ST, :])
            nc.gpsimd.dma_start(scratch[N_ST:2 * N_ST, :],
                                ftmp[DTR + N_ST:DTR + 2 * N_ST, :])
            nc.gpsimd.dma_start(scratch[2 * N_ST:3 * N_ST, :],
                                xdb[DTR + 2 * N_ST:DTR + 3 * N_ST, :])
            nc.gpsimd.dma_start(scratch[3 * N_ST:4 * N_ST, :],
                                ftmp[DTR + 3 * N_ST:DTR + 4 * N_ST, :])

        # ============ phase 2: bidirectional selective scan ============
        with (
            tc.tile_pool(name="ypsum", bufs=1, space="PSUM") as yps,
            tc.tile_pool(name="scanp", bufs=3) as sp,
            tc.tile_pool(name="epool", bufs=2) as ep,
            tc.tile_pool(name="bcp", bufs=4) as bcp,
        ):
            ypt = {}
            for di in range(2):
                for t in range(NDT):
                    for lh in range(2):
                        ypt[(di, t, lh)] = yps.tile(
                            [P, 512], dt.float32,
                            name=f"y{di}{t}{lh}", tag=f"y{di}{t}{lh}")
            NG = N_ST // 2   # n-pair groups
            for g8 in range(NG):
                bcast = []
                for bi in range(4):
                    bt = bcp.tile([P, 2 * L], ST, name=f"bc{bi}", tag=f"bc{bi}")
                    src = scratch[bi * N_ST + 2 * g8:bi * N_ST + 2 * g8 + 2, :]
                    nc.sync.dma_start(
                        bt[:].rearrange("p (g l) -> p g l", g=2),
                        src.unsqueeze(0).broadcast_to([P, 2, L]))
                    bcast.append(bt)
                for t in range(NDT):
                    # decay tiles for the n-pair: separate fwd/bwd boundary zeros
                    Ef = ep.tile([P, 2 * L], dt.float32, name="Ef", tag="Ef")
                    Eb = ep.tile([P, 2 * L], dt.float32, name="Eb", tag="Eb")
                    for half in range(2):
                        acol = a_sb[t][:, 2 * g8 + half:2 * g8 + half + 1]
                        nc.scalar.activation(Ef[:, half * L:(half + 1) * L],
                                             delta[t][:], AF.Exp, bias=0.0, scale=acol)
                        nc.scalar.activation(Eb[:, half * L:(half + 1) * L],
                                             delta[t][:], AF.Exp, bias=0.0, scale=acol)
                    # reset the recurrence at the n-block boundary
                    nc.scalar.mul(Ef[:, L:L + 1], Ef[:, L:L + 1], 0.0)
                    nc.scalar.mul(Eb[:, L - 1:L], Eb[:, L - 1:L], 0.0)
                    zrep = zu[t][:].unsqueeze(1).broadcast_to([P, 2, L])
                    for di in range(2):
                        dbu = sp.tile([P, 2 * L], ST, name="dbu", tag="dbu")
                        nc.vector.tensor_tensor(
                            dbu[:].rearrange("p (g l) -> p g l", g=2), zrep,
                            bcast[di][:].rearrange("p (g l) -> p g l", g=2), Alu.mult)
                        h = sp.tile([P, 2 * L], ST, name="h", tag="h")
                        if di == 0:
                            nc.vector.tensor_tensor_scan(h[:], Ef[:], dbu[:], 0.0,
                                                         Alu.mult, Alu.add)
                        else:
                            nc.vector.tensor_tensor_scan(h[:, ::-1], Eb[:, ::-1],
                                                         dbu[:, ::-1], 0.0,
                                                         Alu.mult, Alu.add)
                        g = sp.tile([P, 2 * L], ST, name="g", tag="g")
                        mul2 = nc.gpsimd if t == 1 else nc.vector
                        mul2.tensor_tensor(g[:], h[:], bcast[2 + di][:], Alu.mult)
                        for nb in range(2):
                            for lh in range(2):
                                nc.tensor.matmul(
                                    ypt[(di, t, lh)][:], ident16[:],
                                    g[:, nb * L + lh * 512:nb * L + (lh + 1) * 512],
                                    start=(g8 == 0 and nb == 0),
                                    stop=(g8 == NG - 1 and nb == 1))

            # gating: gated = (xs * D + y_scan) * silu(res)
            for di in range(2):
                for t in range(NDT):
                    gt = gated[(di, t)]
                    gtmp = sp.tile([P, L], dt.float32, name="gtmp", tag="gtmp")
                    for lh in range(2):
                        nc.vector.scalar_tensor_tensor(
                            gtmp[:, lh * 512:(lh + 1) * 512],
                            xs[t][:, lh * 512:(lh + 1) * 512],
                            dpar_sb[t][:], ypt[(di, t, lh)][:],
                            Alu.mult, Alu.add)
                    nc.vector.tensor_mul(gt[:], gtmp[:], sres[t][:])

        # ============ phase 3: output projection (bf16) ============
        with (
            tc.tile_pool(name="ops", bufs=3, space="PSUM") as ops,
            tc.tile_pool(name="osb", bufs=3) as osb,
        ):
            for c in range(NLC):
                om = ops.tile([P, D_IN], dt.float32, name="om", tag="om")
                idx = 0
                for di in range(2):
                    for t in range(NDT):
                        nc.tensor.matmul(om[:], gated[(di, t)][:, c * P:(c + 1) * P],
                                         w_out_sb[di * NDT + t][:],
                                         start=(idx == 0), stop=(idx == 3))
                        idx += 1
                ot = osb.tile([P, D_IN], dt.float32, name="ot", tag="ot")
                nc.vector.tensor_copy(ot[:], om[:])
                nc.sync.dma_start(out_d[c * P:(c + 1) * P, :], ot[:])

    nc.finalize()
    return nc


def _shard_inputs(inputs, W_in, conv_k, conv_b, W_x, W_dt, b_dt, A_log, D_param, W_out):
    f32 = np.float32
    inputs = np.asarray(inputs, f32)
    W_in = np.asarray(W_in, f32)
    ck = np.asarray(conv_k, f32).reshape(D_CONV, D_INT)
    cb = np.asarray(conv_b, f32)
    W_x = np.asarray(W_x, f32)
    W_dt = np.asarray(W_dt, f32)
    b_dt = np.asarray(b_dt, f32)
    A = -np.exp(np.asarray(A_log, f32))
    D_param = np.asarray(D_param, f32)
    W_out = np.asarray(W_out, f32)

    in_maps = []
    for core in range(N_CORES):
        b, dh = divmod(core, 2)
        perm = np.concatenate([np.arange(dh * DH, (dh + 1) * DH),
                               np.arange((1 - dh) * DH, (2 - dh) * DH)])
        half = perm[:DH]
        w_in_p = np.concatenate([W_in[:, :D_INT][:, perm], W_in[:, D_INT:][:, perm]],
                                axis=1)
        ckp = ck[:, perm]                      # [4, 512]
        ckd = np.zeros((4, D_CONV, P, P), f32)
        for t in range(4):
            for w in range(D_CONV):
                np.fill_diagonal(ckd[t, w], ckp[w, t * P:(t + 1) * P])
        w_out4 = np.stack([
            W_out[half[0:P]], W_out[half[P:2 * P]],
            W_out[D_INT + half[0:P]], W_out[D_INT + half[P:2 * P]],
        ])
        in_maps.append({
            "inpT": np.ascontiguousarray(inputs[b].T),
            "w_in": np.ascontiguousarray(w_in_p),
            "ckd": ckd,
            "cb": np.ascontiguousarray(cb[perm][:, None]),
            "w_x": np.ascontiguousarray(W_x[perm]),
            "w_dt": np.ascontiguousarray(W_dt[:, half]),
            "bdt": np.ascontiguousarray(b_dt[half][:, None]),
            "a": np.ascontiguousarray(A[half]),
            "dpar": np.ascontiguousarray(D_param[half][:, None]),
            "w_out": np.ascontiguousarray(w_out4),
        })
    return in_maps


LAST_EXEC_NS = None


def kernel(**inputs):
    global LAST_EXEC_NS
    import os
    from concourse.bass_utils import run_bass_kernel_spmd

    if "nc" not in _cache:
        _cache["nc"] = _build_program()
    nc = _cache["nc"]
    in_maps = _shard_inputs(**inputs)
    trace = bool(int(os.environ.get("BIMAMBA_TRACE", "0")))
    res = run_bass_kernel_spmd(nc, in_maps, core_ids=list(range(N_CORES)), trace=trace)
    _cache["last_res"] = res
    LAST_EXEC_NS = res.exec_time_ns
    out = np.zeros((B_SZ, L, D_IN), np.float32)
    for b in range(B_SZ):
        out[b] = res.results[2 * b]["out_part"] + res.results[2 * b + 1]["out_part"]
    return out

